# revision 10
# baseline (speedup 1.0000x reference)
"""Multihead attention (B=4, S=2048, D=1024, H=16, Hd=64) on 8 trn2 cores.

Sharding: core c owns batch b = c//2 and heads [(c%2)*8, (c%2)*8+8).
Each core computes q/k/v projections for its 8 heads, attention, and the
partial output projection restricted to its heads' context features.
Host adds the two partials per batch element (+ bo).

v3: bf16 operands (PE 1 cyc/col), softmax exp on ACT from [128,1024]
two-bank PSUM tiles, and — critically — a fully software-pipelined
emission order: the q/k/v projections of head-pair hp+1 are interleaved
into head-pair hp's attention chunks, and the output projection is
interleaved into the last head-pair's chunks. This keeps the PE >90%
busy so the HAM clock gate stays at 2.4 GHz (a 60%-duty attention phase
measurably re-throttles the PE to 1.2 GHz and doubles the tensor time).

Layout: inputs pre-transposed (xT: [D,S]); q,k produced transposed
([hd, tok]); scores S^T = K @ Q^T so the exp output A^T is the AV
stationary operand; AV appends a ones-column to V so out[q,64] is the
softmax denominator (normalize = per-partition reciprocal+scale); ctx is
PE-transposed into ctx^T for the output projection. Softmax skips
max-subtraction (scores ~N(0,1), exp safe in fp32). The four AV chains
of a head share one PSUM bank: only the first issues start=True (which
clears has_written for the whole bank); the others' first matmul relies
on overwrite-where-clear.
"""

import numpy as np
import ml_dtypes

B, S, D = 4, 2048, 1024
H, HD = 16, 64
HPC = 8              # heads per core
HF = HPC * HD        # 512 head-features per core
NCORES = 8
QC = 512             # chunk (moving free dim) for proj and attention
NQC = S // QC        # 4
KT = S // 128        # 16 k-token tiles
PT = 128

_cache = {}


def _build_nc(reps=1):
    from contextlib import ExitStack

    import concourse.mybir as mybir
    import concourse.tile as tile
    from concourse import bacc

    f32 = mybir.dt.float32
    bf16 = mybir.dt.bfloat16
    nc = bacc.Bacc()

    xqT = nc.declare_dram_parameter("xqT", [D, S], bf16, isOutput=False)
    xkT = nc.declare_dram_parameter("xkT", [D, S], bf16, isOutput=False)
    xvT = nc.declare_dram_parameter("xvT", [D, S], bf16, isOutput=False)
    wqT = nc.declare_dram_parameter("wqT", [D, HF], bf16, isOutput=False)
    wkT = nc.declare_dram_parameter("wkT", [D, HF], bf16, isOutput=False)
    wvT = nc.declare_dram_parameter("wvT", [D, HF], bf16, isOutput=False)
    woT = nc.declare_dram_parameter("woT", [HF, D], bf16, isOutput=False)
    bq = nc.declare_dram_parameter("bq", [HF], f32, isOutput=False)
    bk = nc.declare_dram_parameter("bk", [HF], f32, isOutput=False)
    bv = nc.declare_dram_parameter("bv", [HF], f32, isOutput=False)
    out = nc.declare_dram_parameter("out", [S, D], bf16, isOutput=True)
    identd = nc.declare_dram_parameter("ident", [PT, PT], bf16, isOutput=False)

    DKT = D // PT  # 8 feature k-tiles for projections

    with tile.TileContext(nc) as tc, ExitStack() as ctx:
        persist = ctx.enter_context(tc.tile_pool(name="persist", bufs=1))
        xpool = ctx.enter_context(tc.tile_pool(name="xp", bufs=2))
        auxps = ctx.enter_context(tc.tile_pool(name="aux", bufs=2, space="PSUM"))

        qT = [persist.tile([PT, S], bf16, name=f"qT{i}", tag=f"qT{i}") for i in range(4)]
        kT = [persist.tile([PT, S], bf16, name=f"kT{i}", tag=f"kT{i}") for i in range(4)]
        vst = [persist.tile([PT, HPC * (HD + 1)], bf16, name=f"v{t}", tag=f"v{t}")
               for t in range(KT)]
        ctxT = [persist.tile([PT, S], bf16, name=f"ctxT{i}", tag=f"ctxT{i}") for i in range(4)]
        bvb = persist.tile([PT, HF], f32, tag="bvb")
        ident = persist.tile([PT, PT], bf16, tag="ident")
        nc.sync.dma_start(ident[:], identd[:])

        for t in range(KT):
            v3 = vst[t].rearrange("p (h c) -> p h c", c=HD + 1)
            nc.vector.memset(v3[:, :, HD : HD + 1], 1.0)

        # all weights + biases resident up front
        import concourse.bass as bass
        bv_ap = bv[:]
        bv_bc_src = bass.AP(tensor=bv_ap.tensor, offset=bv_ap.offset,
                            ap=[[0, PT], [1, HF]])
        nc.sync.dma_start(bvb[:], bv_bc_src)

        # q/k weights + biases first so the first projection units can start
        # after ~2MB of DMA; wv/wo stream in behind them.
        btq = [persist.tile([PT, 1], f32, name=f"btq{m}", tag=f"btq{m}") for m in range(4)]
        btk = [persist.tile([PT, 1], f32, name=f"btk{m}", tag=f"btk{m}") for m in range(4)]
        for m in range(4):
            nc.sync.dma_start(btq[m][:], bq[m * PT : (m + 1) * PT].rearrange("(p o) -> p o", o=1))
            nc.sync.dma_start(btk[m][:], bk[m * PT : (m + 1) * PT].rearrange("(p o) -> p o", o=1))
        wq = [persist.tile([PT, HF], bf16, name=f"wq{k}", tag=f"wq{k}") for k in range(DKT)]
        wk = [persist.tile([PT, HF], bf16, name=f"wk{k}", tag=f"wk{k}") for k in range(DKT)]
        wv = [persist.tile([PT, HF], bf16, name=f"wv{k}", tag=f"wv{k}") for k in range(DKT)]
        wo = [persist.tile([PT, D], bf16, name=f"wo{i}", tag=f"wo{i}") for i in range(4)]
        for k in range(DKT):
            nc.sync.dma_start(wq[k][:], wqT[k * PT : (k + 1) * PT, :])
            nc.sync.dma_start(wk[k][:], wkT[k * PT : (k + 1) * PT, :])

        # ---- projection work units (emitted interleaved) -----------------
        # One unit = one 512-token chunk of one projection for one head-pair
        # (~4k PE cycles). 12 units per head-pair.
        def emit_proj_unit(which, hp, c):
            xT_d = {"q": xqT, "k": xkT, "v": xvT}[which]
            xt = xpool.tile([PT, DKT, QC], bf16, name=f"x{which}", tag=f"x{which}")
            src = bass.AP(
                tensor=xT_d[:].tensor, offset=c * QC,
                ap=[[S, PT], [PT * S, DKT], [1, QC]],
            )
            nc.sync.dma_start(xt[:], src)
            if which in ("q", "k"):
                wt = wq if which == "q" else wk
                dstT = qT if which == "q" else kT
                bt = btq if which == "q" else btk
                ps = auxps.tile([PT, QC], f32, tag="ps")
                for k in range(DKT):
                    nc.tensor.matmul(
                        ps[:],
                        lhsT=wt[k][:, hp * PT : (hp + 1) * PT],
                        rhs=xt[:, k, :],
                        start=(k == 0),
                        stop=(k == DKT - 1),
                    )
                nc.vector.tensor_scalar_add(
                    dstT[hp][:, c * QC : (c + 1) * QC], ps[:], bt[hp][:]
                )
            else:
                for mt in range(4):  # tok-tiles in chunk
                    t = c * 4 + mt
                    ps = auxps.tile([PT, PT], f32, tag="ps")
                    for k in range(DKT):
                        nc.tensor.matmul(
                            ps[:],
                            lhsT=xt[:, k, mt * PT : (mt + 1) * PT],
                            rhs=wv[k][:, hp * PT : (hp + 1) * PT],
                            start=(k == 0),
                            stop=(k == DKT - 1),
                        )
                    v3 = vst[t].rearrange("p (h c) -> p h c", c=HD + 1)
                    nc.vector.tensor_add(
                        v3[:, 2 * hp : 2 * hp + 2, 0:HD],
                        ps[:].rearrange("p (h c) -> p h c", c=HD),
                        bvb[:, hp * PT : (hp + 1) * PT].rearrange("p (h c) -> p h c", c=HD),
                    )

        def proj_units(hp):
            return [(w, hp, c) for c in range(NQC) for w in ("q", "k", "v")]

        # ---- output projection per token-chunk (emitted during hp=3) -----
        def emit_oproj_chunk(c):
            for mt in range(4):
                tt = c * 4 + mt
                for nch in range(2):
                    ps = auxps.tile([PT, QC], f32, tag="ps")
                    for i in range(4):
                        nc.tensor.matmul(
                            ps[:],
                            lhsT=ctxT[i][:, tt * PT : (tt + 1) * PT],
                            rhs=wo[i][:, nch * QC : (nch + 1) * QC],
                            start=(i == 0),
                            stop=(i == 3),
                        )
                    ot = xpool.tile([PT, QC], bf16, name="ot", tag="ot")
                    nc.vector.tensor_copy(ot[:], ps[:])
                    nc.sync.dma_start(
                        out[tt * PT : (tt + 1) * PT, nch * QC : (nch + 1) * QC], ot[:]
                    )

        # ---- fill: projections for head-pair 0 ---------------------------
        # q/k units first (their weights are already queued); wv streams
        # during the q/k matmuls, wo during early attention.
        for c in range(NQC):
            emit_proj_unit("q", 0, c)
            emit_proj_unit("k", 0, c)
            if c == 0:
                for k in range(DKT):
                    nc.sync.dma_start(wv[k][:], wvT[k * PT : (k + 1) * PT, :])
        for c in range(NQC):
            emit_proj_unit("v", 0, c)
        for i in range(4):
            nc.sync.dma_start(wo[i][:], woT[i * PT : (i + 1) * PT, :])

        # ---- attention, software-pipelined -------------------------------
        with tc.tile_pool(name="at", bufs=1) as atpool, \
             tc.tile_pool(name="nrm", bufs=2) as nrmpool, \
             tc.tile_pool(name="cs", bufs=1) as cspool, \
             tc.tile_pool(name="st", bufs=2, space="PSUM") as stpool, \
             tc.tile_pool(name="av", bufs=1, space="PSUM") as avpool:

            for hp in range(4):
                filler = proj_units(hp + 1) if hp < 4 - 1 else []
                fi = 0  # next filler unit
                cs = [cspool.tile([PT, PT], bf16, name=f"cs{t}", tag=f"cs{t}")
                      for t in range(KT)]
                for c in range(NQC):
                    # scores + exp stream continuously; the AV batch for score
                    # group (e,g) is emitted ~2 groups later so the PE never
                    # sits in a long AV-only block starving the ACT engine.
                    # Proj units of the next head-pair fill remaining PE slack.
                    avps = {}
                    for e in range(2):
                        avps[e] = avpool.tile([PT, 4, HD + 1], f32,
                                              name=f"av{e}", tag=f"av{e}")

                    def emit_av_batch(e, g, at_tile, hp=hp):
                        # AV chain steps kt=2g, 2g+1; 4 chains of a head share
                        # one PSUM bank: only (kt==0, qt==0) issues start=True
                        # (whole-bank has_written clear).
                        h = 2 * hp + e
                        for j in range(2):
                            kt = 2 * g + j
                            for qt in range(4):
                                nc.tensor.matmul(
                                    avps[e][:, qt, :],
                                    lhsT=at_tile[:, j * QC + qt * PT :
                                                 j * QC + (qt + 1) * PT],
                                    rhs=vst[kt][:, h * (HD + 1) : (h + 1) * (HD + 1)],
                                    start=(kt == 0 and qt == 0),
                                    stop=(kt == KT - 1),
                                )

                    pending = []
                    n_groups = 0
                    for e in range(2):
                        for g in range(KT // 2):
                            stp = stpool.tile([PT, 2 * QC], f32, name="stp", tag="st")
                            for j in range(2):
                                kt = 2 * g + j
                                nc.tensor.matmul(
                                    stp[:, j * QC : (j + 1) * QC],
                                    lhsT=kT[hp][e * HD : (e + 1) * HD,
                                                kt * PT : (kt + 1) * PT],
                                    rhs=qT[hp][e * HD : (e + 1) * HD,
                                               c * QC : (c + 1) * QC],
                                    start=True,
                                    stop=True,
                                )
                            a = atpool.tile([PT, 2 * QC], bf16,
                                            name=f"at{e}_{g}", tag=f"at{e}_{g}")
                            nc.scalar.activation(
                                a[:], stp[:],
                                mybir.ActivationFunctionType.Exp,
                                scale=1.0 / np.sqrt(HD),
                            )
                            pending.append((e, g, a))
                            if len(pending) >= 2:
                                emit_av_batch(*pending.pop(0))
                            n_groups += 1
                            if fi < len(filler) and n_groups % 5 == 0:
                                emit_proj_unit(*filler[fi])
                                fi += 1
                    for item in pending:
                        emit_av_batch(*item)
                    for e in range(2):
                        for qt in range(4):
                            linv = nrmpool.tile([PT, 1], f32, tag="linv")
                            nc.vector.reciprocal(linv[:], avps[e][:, qt, HD : HD + 1])
                            nc.vector.tensor_scalar_mul(
                                cs[c * 4 + qt][:, e * HD : (e + 1) * HD],
                                avps[e][:, qt, 0:HD],
                                linv[:],
                            )
                    # per-chunk transposes into ctx^T
                    for mt in range(4):
                        tt = c * 4 + mt
                        tp = stpool.tile([PT, PT], bf16, name="tp", tag="st")
                        nc.tensor.transpose(tp[:], cs[tt][:], ident[:])
                        nc.vector.tensor_copy(ctxT[hp][:, tt * PT : (tt + 1) * PT], tp[:])
                    if hp == 3:
                        emit_oproj_chunk(c)
                # leftover filler units (shouldn't happen, but be safe)
                while fi < len(filler):
                    emit_proj_unit(*filler[fi])
                    fi += 1

    nc.compile()
    return nc


def make_in_maps(inputs):
    bf = ml_dtypes.bfloat16
    q = np.asarray(inputs["query"], np.float32)
    k = np.asarray(inputs["key"], np.float32)
    v = np.asarray(inputs["value"], np.float32)
    Wq, Wk, Wv, Wo = (np.asarray(inputs[n], np.float32) for n in ("Wq", "Wk", "Wv", "Wo"))
    bq, bk, bv, bo = (np.asarray(inputs[n], np.float32) for n in ("bq", "bk", "bv", "bo"))

    xqTb = [np.ascontiguousarray(q[b].T).astype(bf) for b in range(B)]
    xkTb = [np.ascontiguousarray(k[b].T).astype(bf) for b in range(B)]
    xvTb = [np.ascontiguousarray(v[b].T).astype(bf) for b in range(B)]

    in_maps = []
    for c in range(NCORES):
        b, half = c // 2, c % 2
        fs = slice(half * HF, (half + 1) * HF)
        in_maps.append({
            "xqT": xqTb[b],
            "xkT": xkTb[b],
            "xvT": xvTb[b],
            "wqT": np.ascontiguousarray(Wq[fs, :].T).astype(bf),
            "wkT": np.ascontiguousarray(Wk[fs, :].T).astype(bf),
            "wvT": np.ascontiguousarray(Wv[fs, :].T).astype(bf),
            "woT": np.ascontiguousarray(Wo[:, fs].T).astype(bf),
            "bq": np.ascontiguousarray(bq[fs]),
            "bk": np.ascontiguousarray(bk[fs]),
            "bv": np.ascontiguousarray(bv[fs]),
            "ident": np.eye(PT, dtype=bf),
        })
    return in_maps


def kernel(**inputs):
    from concourse.bass_utils import run_bass_kernel_spmd

    if "nc" not in _cache:
        _cache["nc"] = _build_nc()
    nc = _cache["nc"]

    in_maps = make_in_maps(inputs)
    res = run_bass_kernel_spmd(nc, in_maps, list(range(NCORES)))
    _cache["last_result"] = res

    bo = np.asarray(inputs["bo"], np.float32)
    out = np.empty((B, S, D), np.float32)
    for b in range(B):
        out[b] = (res.results[2 * b]["out"].astype(np.float32)
                  + res.results[2 * b + 1]["out"].astype(np.float32) + bo)
    return out


# revision 11
# speedup vs baseline: 1.0412x; 1.0412x over previous
"""Multihead attention (B=4, S=2048, D=1024, H=16, Hd=64) on 8 trn2 cores.

Sharding: core c owns batch b = c//2 and heads [(c%2)*8, (c%2)*8+8).
Each core computes q/k/v projections for its 8 heads, attention, and the
partial output projection restricted to its heads' context features.
Host adds the two partials per batch element (+ bo).

v3: bf16 operands (PE 1 cyc/col), softmax exp on ACT from [128,1024]
two-bank PSUM tiles, and — critically — a fully software-pipelined
emission order: the q/k/v projections of head-pair hp+1 are interleaved
into head-pair hp's attention chunks, and the output projection is
interleaved into the last head-pair's chunks. This keeps the PE >90%
busy so the HAM clock gate stays at 2.4 GHz (a 60%-duty attention phase
measurably re-throttles the PE to 1.2 GHz and doubles the tensor time).

Layout: inputs pre-transposed (xT: [D,S]); q,k produced transposed
([hd, tok]); scores S^T = K @ Q^T so the exp output A^T is the AV
stationary operand; AV appends a ones-column to V so out[q,64] is the
softmax denominator (normalize = per-partition reciprocal+scale); ctx is
PE-transposed into ctx^T for the output projection. Softmax skips
max-subtraction (scores ~N(0,1), exp safe in fp32). The four AV chains
of a head share one PSUM bank: only the first issues start=True (which
clears has_written for the whole bank); the others' first matmul relies
on overwrite-where-clear.
"""

import numpy as np
import ml_dtypes

B, S, D = 4, 2048, 1024
H, HD = 16, 64
HPC = 8              # heads per core
HF = HPC * HD        # 512 head-features per core
NCORES = 8
QC = 512             # chunk (moving free dim) for proj and attention
NQC = S // QC        # 4
KT = S // 128        # 16 k-token tiles
PT = 128

_cache = {}


def _build_nc(reps=1):
    from contextlib import ExitStack

    import concourse.mybir as mybir
    import concourse.tile as tile
    from concourse import bacc

    f32 = mybir.dt.float32
    bf16 = mybir.dt.bfloat16
    nc = bacc.Bacc()

    xqT = nc.declare_dram_parameter("xqT", [D, S], bf16, isOutput=False)
    xkT = nc.declare_dram_parameter("xkT", [D, S], bf16, isOutput=False)
    xvT = nc.declare_dram_parameter("xvT", [D, S], bf16, isOutput=False)
    wqT = nc.declare_dram_parameter("wqT", [D, HF], bf16, isOutput=False)
    wkT = nc.declare_dram_parameter("wkT", [D, HF], bf16, isOutput=False)
    wvT = nc.declare_dram_parameter("wvT", [D, HF], bf16, isOutput=False)
    woT = nc.declare_dram_parameter("woT", [HF, D], bf16, isOutput=False)
    bq = nc.declare_dram_parameter("bq", [HF], f32, isOutput=False)
    bk = nc.declare_dram_parameter("bk", [HF], f32, isOutput=False)
    bv = nc.declare_dram_parameter("bv", [HF], f32, isOutput=False)
    out = nc.declare_dram_parameter("out", [S, D], bf16, isOutput=True)
    identd = nc.declare_dram_parameter("ident", [PT, PT], bf16, isOutput=False)

    DKT = D // PT  # 8 feature k-tiles for projections

    with tile.TileContext(nc) as tc, ExitStack() as ctx:
        persist = ctx.enter_context(tc.tile_pool(name="persist", bufs=1))
        xpool = ctx.enter_context(tc.tile_pool(name="xp", bufs=2))
        auxps = ctx.enter_context(tc.tile_pool(name="aux", bufs=2, space="PSUM"))

        qT = [persist.tile([PT, S], bf16, name=f"qT{i}", tag=f"qT{i}") for i in range(4)]
        kT = [persist.tile([PT, S], bf16, name=f"kT{i}", tag=f"kT{i}") for i in range(4)]
        vst = [persist.tile([PT, HPC * (HD + 1)], bf16, name=f"v{t}", tag=f"v{t}")
               for t in range(KT)]
        ctxT = [persist.tile([PT, S], bf16, name=f"ctxT{i}", tag=f"ctxT{i}") for i in range(4)]
        bvb = persist.tile([PT, HF], f32, tag="bvb")
        ident = persist.tile([PT, PT], bf16, tag="ident")
        nc.sync.dma_start(ident[:], identd[:])

        for t in range(KT):
            v3 = vst[t].rearrange("p (h c) -> p h c", c=HD + 1)
            nc.vector.memset(v3[:, :, HD : HD + 1], 1.0)

        # all weights + biases resident up front
        import concourse.bass as bass
        bv_ap = bv[:]
        bv_bc_src = bass.AP(tensor=bv_ap.tensor, offset=bv_ap.offset,
                            ap=[[0, PT], [1, HF]])
        nc.sync.dma_start(bvb[:], bv_bc_src)

        # q/k weights + biases first so the first projection units can start
        # after ~2MB of DMA; wv/wo stream in behind them.
        btq = [persist.tile([PT, 1], f32, name=f"btq{m}", tag=f"btq{m}") for m in range(4)]
        btk = [persist.tile([PT, 1], f32, name=f"btk{m}", tag=f"btk{m}") for m in range(4)]
        for m in range(4):
            nc.sync.dma_start(btq[m][:], bq[m * PT : (m + 1) * PT].rearrange("(p o) -> p o", o=1))
            nc.sync.dma_start(btk[m][:], bk[m * PT : (m + 1) * PT].rearrange("(p o) -> p o", o=1))
        wq = [persist.tile([PT, HF], bf16, name=f"wq{k}", tag=f"wq{k}") for k in range(DKT)]
        wk = [persist.tile([PT, HF], bf16, name=f"wk{k}", tag=f"wk{k}") for k in range(DKT)]
        wv = [persist.tile([PT, HF], bf16, name=f"wv{k}", tag=f"wv{k}") for k in range(DKT)]
        wo = [persist.tile([PT, D], bf16, name=f"wo{i}", tag=f"wo{i}") for i in range(4)]
        for k in range(DKT):
            nc.sync.dma_start(wq[k][:], wqT[k * PT : (k + 1) * PT, :])
            nc.sync.dma_start(wk[k][:], wkT[k * PT : (k + 1) * PT, :])

        # ---- projection work units (emitted interleaved) -----------------
        # One unit = one 512-token chunk of one projection for one head-pair
        # (~4k PE cycles). 12 units per head-pair.
        def emit_proj_unit(which, hp, c):
            xT_d = {"q": xqT, "k": xkT, "v": xvT}[which]
            xt = xpool.tile([PT, DKT, QC], bf16, name=f"x{which}", tag=f"x{which}")
            src = bass.AP(
                tensor=xT_d[:].tensor, offset=c * QC,
                ap=[[S, PT], [PT * S, DKT], [1, QC]],
            )
            nc.sync.dma_start(xt[:], src)
            if which in ("q", "k"):
                wt = wq if which == "q" else wk
                dstT = qT if which == "q" else kT
                bt = btq if which == "q" else btk
                ps = auxps.tile([PT, QC], f32, tag="ps")
                for k in range(DKT):
                    nc.tensor.matmul(
                        ps[:],
                        lhsT=wt[k][:, hp * PT : (hp + 1) * PT],
                        rhs=xt[:, k, :],
                        start=(k == 0),
                        stop=(k == DKT - 1),
                    )
                nc.vector.tensor_scalar_add(
                    dstT[hp][:, c * QC : (c + 1) * QC], ps[:], bt[hp][:]
                )
            else:
                for mt in range(4):  # tok-tiles in chunk
                    t = c * 4 + mt
                    ps = auxps.tile([PT, PT], f32, tag="ps")
                    for k in range(DKT):
                        nc.tensor.matmul(
                            ps[:],
                            lhsT=xt[:, k, mt * PT : (mt + 1) * PT],
                            rhs=wv[k][:, hp * PT : (hp + 1) * PT],
                            start=(k == 0),
                            stop=(k == DKT - 1),
                        )
                    v3 = vst[t].rearrange("p (h c) -> p h c", c=HD + 1)
                    nc.vector.tensor_add(
                        v3[:, 2 * hp : 2 * hp + 2, 0:HD],
                        ps[:].rearrange("p (h c) -> p h c", c=HD),
                        bvb[:, hp * PT : (hp + 1) * PT].rearrange("p (h c) -> p h c", c=HD),
                    )

        def proj_units(hp):
            return [(w, hp, c) for c in range(NQC) for w in ("q", "k", "v")]

        # ---- output projection per token-chunk (emitted during hp=3) -----
        def emit_oproj_chunk(c):
            for mt in range(4):
                tt = c * 4 + mt
                for nch in range(2):
                    ps = auxps.tile([PT, QC], f32, tag="ps")
                    for i in range(4):
                        nc.tensor.matmul(
                            ps[:],
                            lhsT=ctxT[i][:, tt * PT : (tt + 1) * PT],
                            rhs=wo[i][:, nch * QC : (nch + 1) * QC],
                            start=(i == 0),
                            stop=(i == 3),
                        )
                    ot = xpool.tile([PT, QC], bf16, name="ot", tag="ot")
                    nc.vector.tensor_copy(ot[:], ps[:])
                    nc.sync.dma_start(
                        out[tt * PT : (tt + 1) * PT, nch * QC : (nch + 1) * QC], ot[:]
                    )

        # ---- fill: projections for head-pair 0 ---------------------------
        # q/k units first (their weights are already queued); wv streams
        # during the q/k matmuls, wo during early attention.
        for c in range(NQC):
            emit_proj_unit("q", 0, c)
            emit_proj_unit("k", 0, c)
            if c == 0:
                for k in range(DKT):
                    nc.sync.dma_start(wv[k][:], wvT[k * PT : (k + 1) * PT, :])
        for c in range(NQC):
            emit_proj_unit("v", 0, c)
        for i in range(4):
            nc.sync.dma_start(wo[i][:], woT[i * PT : (i + 1) * PT, :])

        # ---- attention, software-pipelined -------------------------------
        with tc.tile_pool(name="at", bufs=1) as atpool, \
             tc.tile_pool(name="nrm", bufs=2) as nrmpool, \
             tc.tile_pool(name="cs", bufs=1) as cspool, \
             tc.tile_pool(name="st", bufs=2, space="PSUM") as stpool, \
             tc.tile_pool(name="av", bufs=1, space="PSUM") as avpool:

            for hp in range(4):
                filler = proj_units(hp + 1) if hp < 4 - 1 else []
                fi = 0  # next filler unit
                cs = [cspool.tile([PT, PT], bf16, name=f"cs{t}", tag=f"cs{t}")
                      for t in range(KT)]
                for c in range(NQC):
                    # scores + exp; proj units of the next head-pair are
                    # emitted between score groups to keep the PE dense.
                    at = {}
                    n_groups = 0
                    for e in range(2):
                        for g in range(KT // 2):
                            stp = stpool.tile([PT, 2 * QC], f32, name="stp", tag="st")
                            for j in range(2):
                                kt = 2 * g + j
                                nc.tensor.matmul(
                                    stp[:, j * QC : (j + 1) * QC],
                                    lhsT=kT[hp][e * HD : (e + 1) * HD,
                                                kt * PT : (kt + 1) * PT],
                                    rhs=qT[hp][e * HD : (e + 1) * HD,
                                               c * QC : (c + 1) * QC],
                                    start=True,
                                    stop=True,
                                )
                            a = atpool.tile([PT, 2 * QC], bf16,
                                            name=f"at{e}_{g}", tag=f"at{e}_{g}")
                            nc.scalar.activation(
                                a[:], stp[:],
                                mybir.ActivationFunctionType.Exp,
                                scale=1.0 / np.sqrt(HD),
                            )
                            at[e, g] = a
                            n_groups += 1
                            # ~3 proj units per chunk, spread across groups
                            if fi < len(filler) and n_groups % 5 == 0:
                                emit_proj_unit(*filler[fi])
                                fi += 1
                    # AV chains; 4 chains of a head share one PSUM bank, so
                    # only the qt=0 chain issues start=True (bank clear).
                    avps = {}
                    for e in range(2):
                        avps[e] = avpool.tile([PT, 4, HD + 1], f32,
                                              name=f"av{e}", tag=f"av{e}")
                    for kt in range(KT):
                        g, j = kt // 2, kt % 2
                        for e in range(2):
                            h = 2 * hp + e
                            for qt in range(4):
                                nc.tensor.matmul(
                                    avps[e][:, qt, :],
                                    lhsT=at[e, g][:, j * QC + qt * PT :
                                                  j * QC + (qt + 1) * PT],
                                    rhs=vst[kt][:, h * (HD + 1) : (h + 1) * (HD + 1)],
                                    start=(kt == 0 and qt == 0),
                                    stop=(kt == KT - 1),
                                )
                    for e in range(2):
                        for qt in range(4):
                            linv = nrmpool.tile([PT, 1], f32, tag="linv")
                            nc.vector.reciprocal(linv[:], avps[e][:, qt, HD : HD + 1])
                            nc.vector.tensor_scalar_mul(
                                cs[c * 4 + qt][:, e * HD : (e + 1) * HD],
                                avps[e][:, qt, 0:HD],
                                linv[:],
                            )
                    # per-chunk transposes into ctx^T
                    for mt in range(4):
                        tt = c * 4 + mt
                        tp = stpool.tile([PT, PT], bf16, name="tp", tag="st")
                        nc.tensor.transpose(tp[:], cs[tt][:], ident[:])
                        nc.vector.tensor_copy(ctxT[hp][:, tt * PT : (tt + 1) * PT], tp[:])
                    if hp == 3:
                        emit_oproj_chunk(c)
                # leftover filler units (shouldn't happen, but be safe)
                while fi < len(filler):
                    emit_proj_unit(*filler[fi])
                    fi += 1

    nc.compile()
    return nc


def make_in_maps(inputs):
    bf = ml_dtypes.bfloat16
    q = np.asarray(inputs["query"], np.float32)
    k = np.asarray(inputs["key"], np.float32)
    v = np.asarray(inputs["value"], np.float32)
    Wq, Wk, Wv, Wo = (np.asarray(inputs[n], np.float32) for n in ("Wq", "Wk", "Wv", "Wo"))
    bq, bk, bv, bo = (np.asarray(inputs[n], np.float32) for n in ("bq", "bk", "bv", "bo"))

    xqTb = [np.ascontiguousarray(q[b].T).astype(bf) for b in range(B)]
    xkTb = [np.ascontiguousarray(k[b].T).astype(bf) for b in range(B)]
    xvTb = [np.ascontiguousarray(v[b].T).astype(bf) for b in range(B)]

    in_maps = []
    for c in range(NCORES):
        b, half = c // 2, c % 2
        fs = slice(half * HF, (half + 1) * HF)
        in_maps.append({
            "xqT": xqTb[b],
            "xkT": xkTb[b],
            "xvT": xvTb[b],
            "wqT": np.ascontiguousarray(Wq[fs, :].T).astype(bf),
            "wkT": np.ascontiguousarray(Wk[fs, :].T).astype(bf),
            "wvT": np.ascontiguousarray(Wv[fs, :].T).astype(bf),
            "woT": np.ascontiguousarray(Wo[:, fs].T).astype(bf),
            "bq": np.ascontiguousarray(bq[fs]),
            "bk": np.ascontiguousarray(bk[fs]),
            "bv": np.ascontiguousarray(bv[fs]),
            "ident": np.eye(PT, dtype=bf),
        })
    return in_maps


def kernel(**inputs):
    from concourse.bass_utils import run_bass_kernel_spmd

    if "nc" not in _cache:
        _cache["nc"] = _build_nc()
    nc = _cache["nc"]

    in_maps = make_in_maps(inputs)
    res = run_bass_kernel_spmd(nc, in_maps, list(range(NCORES)))
    _cache["last_result"] = res

    bo = np.asarray(inputs["bo"], np.float32)
    out = np.empty((B, S, D), np.float32)
    for b in range(B):
        out[b] = (res.results[2 * b]["out"].astype(np.float32)
                  + res.results[2 * b + 1]["out"].astype(np.float32) + bo)
    return out
